# revision 4
# baseline (speedup 1.0000x reference)
"""Trainium2 Bass kernel for nn_AttentionMechanism (B=4, LQ=1024, ND=4096, D=1024).

Sharding: batch (4) x num_docs (2) -> 8 cores. Core c handles batch c//2 and
doc half c%2 (2048 docs). Each core computes a partial softmax-attention:
  kT = (Wk @ docsT + bk)          [e, n]   (fp32r matmuls, e on partitions)
  qT = (Wq @ queryT + bq)         [e, lq]
  s  = qT.T @ kT                  [lq, n]  per 128-row chunk, PSUM
  m  = rowmax(s); p = exp(s - m); l = rowsum(p)
  num = p @ docs                  [lq, d]
Host merges the two doc-halves per batch with the standard softmax-stat
rescale and divides by l.

All heavy matmuls run as float32r (TF32-like, full PE rate).
"""

import sys

if "/opt/trn_rl_repo" not in sys.path:
    sys.path.insert(0, "/opt/trn_rl_repo")

import numpy as np

import concourse.bass as bass  # noqa: F401  (engine types referenced via nc)
import concourse.mybir as mybir
from concourse import bacc
from concourse.tile import TileContext
from concourse.masks import make_identity
from concourse.bass_utils import run_bass_kernel_spmd

P = 128
B, LQ, ND, D = 4, 1024, 4096, 1024
N2 = ND // 2  # docs per core
EC = D // P  # 8 e-chunks
DC = D // P  # 8 d-chunks
LC = LQ // P  # 8 lq-chunks
NC = N2 // P  # 16 n-chunks
NT = N2 // 512  # 4 n-tiles of 512

F32 = mybir.dt.float32
F32R = mybir.dt.float32r
ACT = mybir.ActivationFunctionType
AX = mybir.AxisListType

_CACHE = {}


def build_nc():
    nc = bacc.Bacc("TRN2", target_bir_lowering=False)

    qT = nc.dram_tensor("qT", [D, LQ], F32, kind="ExternalInput")
    dT = nc.dram_tensor("dT", [D, N2], F32, kind="ExternalInput")
    dn = nc.dram_tensor("dn", [N2, D], F32, kind="ExternalInput")
    wqT = nc.dram_tensor("wqT", [D, D], F32, kind="ExternalInput")
    wkT = nc.dram_tensor("wkT", [D, D], F32, kind="ExternalInput")
    bqc = nc.dram_tensor("bqc", [P, EC], F32, kind="ExternalInput")
    bkc = nc.dram_tensor("bkc", [P, EC], F32, kind="ExternalInput")

    num = nc.dram_tensor("num", [LQ, D], F32, kind="ExternalOutput")
    mx = nc.dram_tensor("mx", [P, LC], F32, kind="ExternalOutput")
    ls = nc.dram_tensor("ls", [P, LC], F32, kind="ExternalOutput")

    qT_r = qT.ap().rearrange("(dc p) l -> p dc l", p=P).bitcast(F32R)
    dT_r = dT.ap().rearrange("(dc p) n -> p dc n", p=P).bitcast(F32R)
    dn_r = dn.ap().rearrange("(nc p) d -> p nc d", p=P).bitcast(F32R)
    wqT_r = wqT.ap().rearrange("(dc p) e -> p dc e", p=P).bitcast(F32R)
    wkT_r = wkT.ap().rearrange("(dc p) e -> p dc e", p=P).bitcast(F32R)

    with TileContext(nc) as tc:
        with (
            tc.tile_pool(name="const", bufs=1) as cpool,
            tc.tile_pool(name="stats", bufs=1) as spool,
            tc.tile_pool(name="kTp", bufs=1) as kT_pool,
        ):
            ident32 = cpool.tile([P, P], F32)
            make_identity(nc, ident32[:])
            ident = cpool.tile([P, P], F32R)
            nc.vector.tensor_copy(ident[:], ident32[:])
            bq_s = cpool.tile([P, EC], F32)
            bk_s = cpool.tile([P, EC], F32)
            nc.sync.dma_start(bq_s[:], bqc.ap())
            nc.sync.dma_start(bk_s[:], bkc.ap())

            mx_all = spool.tile([P, LC], F32)
            ls_all = spool.tile([P, LC], F32)

            kT = kT_pool.tile([P, EC, N2], F32R)  # 64KB/part

            # ---- Phase K: kT[e, n] = Wk @ docsT + bk ----
            with (
                tc.tile_pool(name="pk", bufs=1) as pk,
                tc.tile_pool(name="psk", bufs=8, space="PSUM") as psk,
            ):
                wk_s = pk.tile([P, DC, D], F32R)
                dT_s = pk.tile([P, DC, N2], F32R)
                nc.sync.dma_start(wk_s[:], wkT_r)
                nc.sync.dma_start(dT_s[:], dT_r)
                for ec in range(EC):
                    pss = [psk.tile([P, 512], F32, name="psk") for t in range(NT)]
                    for dc in range(DC):
                        for t in range(NT):
                            nc.tensor.matmul(
                                pss[t][:],
                                wk_s[:, dc, ec * P : (ec + 1) * P],
                                dT_s[:, dc, t * 512 : (t + 1) * 512],
                                start=(dc == 0),
                                stop=(dc == DC - 1),
                            )
                    for t in range(NT):
                        nc.scalar.activation(
                            kT[:, ec, t * 512 : (t + 1) * 512],
                            pss[t][:],
                            ACT.Identity,
                            bias=bk_s[:, ec : ec + 1],
                        )

            with tc.tile_pool(name="qTp", bufs=1) as qTp_pool:
                qTp = qTp_pool.tile([P, EC, LQ], F32R)  # 32KB/part

                # ---- Phase Q: qTp[e, lq] = Wq @ queryT + bq ----
                with (
                    tc.tile_pool(name="pq", bufs=1) as pq,
                    tc.tile_pool(name="psq", bufs=4, space="PSUM") as psq,
                ):
                    wq_s = pq.tile([P, DC, D], F32R)
                    qT_s = pq.tile([P, DC, LQ], F32R)
                    nc.sync.dma_start(wq_s[:], wqT_r)
                    nc.sync.dma_start(qT_s[:], qT_r)
                    for ec in range(EC):
                        pss = [psq.tile([P, 512], F32, name="psq") for t in range(2)]
                        for dc in range(DC):
                            for t in range(2):
                                nc.tensor.matmul(
                                    pss[t][:],
                                    wq_s[:, dc, ec * P : (ec + 1) * P],
                                    qT_s[:, dc, t * 512 : (t + 1) * 512],
                                    start=(dc == 0),
                                    stop=(dc == DC - 1),
                                )
                        for t in range(2):
                            nc.scalar.activation(
                                qTp[:, ec, t * 512 : (t + 1) * 512],
                                pss[t][:],
                                ACT.Identity,
                                bias=bq_s[:, ec : ec + 1],
                            )

                # ---- Phase A: attention per 128-query chunk ----
                with (
                    tc.tile_pool(name="pa", bufs=1) as pa,
                    tc.tile_pool(name="pwork", bufs=2) as pw,
                    tc.tile_pool(name="ps_sc", bufs=1, space="PSUM") as ps_sc,
                    tc.tile_pool(name="ps_av", bufs=1, space="PSUM") as ps_av,
                    tc.tile_pool(name="ps_tp", bufs=2, space="PSUM") as ps_tp,
                ):
                    dn_s = []
                    for i in range(NC):
                        t = pa.tile([P, D], F32R, name=f"dn{i}")
                        nc.sync.dma_start(t[:], dn_r[:, i, :])
                        dn_s.append(t)

                    for lc in range(LC):
                        # scores [128, 2048] in PSUM (4 banks)
                        sc = ps_sc.tile([P, N2], F32, name="sc")
                        for ec in range(EC):
                            for t in range(NT):
                                nc.tensor.matmul(
                                    sc[:, t * 512 : (t + 1) * 512],
                                    qTp[:, ec, lc * P : (lc + 1) * P],
                                    kT[:, ec, t * 512 : (t + 1) * 512],
                                    start=(ec == 0),
                                    stop=(ec == EC - 1),
                                )
                        # row max / -max
                        negmax = pw.tile([P, 1], F32, name="negmax")
                        nc.vector.reduce_max(
                            negmax[:], sc[:], axis=AX.X, negate=True
                        )
                        nc.vector.tensor_scalar_mul(
                            mx_all[:, lc : lc + 1], negmax[:], -1.0
                        )
                        # probs = exp(s - m), l = rowsum
                        probs = pw.tile([P, N2], F32R, name="probs")
                        nc.scalar.activation(
                            probs[:],
                            sc[:],
                            ACT.Exp,
                            bias=negmax[:],
                            accum_out=ls_all[:, lc : lc + 1],
                        )
                        # transpose probs -> probsT [n, lq] chunks
                        probsT = pw.tile([P, NC, P], F32R, name="probsT")
                        for g in range(NC // 4):
                            tp = ps_tp.tile([P, 512], F32R, name="tp")
                            for j in range(4):
                                nn = g * 4 + j
                                nc.tensor.transpose(
                                    tp[:, j * P : (j + 1) * P],
                                    probs[:, nn * P : (nn + 1) * P],
                                    ident[:],
                                )
                            nc.vector.tensor_copy(
                                probsT[:, g * 4 : (g + 1) * 4, :], tp[:]
                            )
                        # AV: num[lq, d] = probs @ dn
                        av = ps_av.tile([P, D], F32, name="av")
                        for nn in range(NC):
                            for dh in range(2):
                                nc.tensor.matmul(
                                    av[:, dh * 512 : (dh + 1) * 512],
                                    probsT[:, nn, :],
                                    dn_s[nn][:, dh * 512 : (dh + 1) * 512],
                                    start=(nn == 0),
                                    stop=(nn == NC - 1),
                                )
                        num_t = pw.tile([P, D], F32, name="num_t")
                        nc.scalar.activation(num_t[:], av[:], ACT.Copy)
                        nc.sync.dma_start(
                            num.ap()[lc * P : (lc + 1) * P, :], num_t[:]
                        )

            nc.sync.dma_start(mx.ap()[:, :], mx_all[:])
            nc.sync.dma_start(ls.ap()[:, :], ls_all[:])

    nc.compile()
    return nc


def _prep_inputs(query, documents, Wq, bq, Wk, bk):
    query = np.asarray(query, dtype=np.float32)
    documents = np.asarray(documents, dtype=np.float32)
    wqT = np.ascontiguousarray(np.asarray(Wq, np.float32).T)
    wkT = np.ascontiguousarray(np.asarray(Wk, np.float32).T)
    bqc = np.ascontiguousarray(np.asarray(bq, np.float32).reshape(EC, P).T)
    bkc = np.ascontiguousarray(np.asarray(bk, np.float32).reshape(EC, P).T)
    in_maps = []
    for b in range(B):
        qT = np.ascontiguousarray(query[b].T)
        for h in range(2):
            d_slice = documents[b, h * N2 : (h + 1) * N2]
            in_maps.append(
                {
                    "qT": qT,
                    "dT": np.ascontiguousarray(d_slice.T),
                    "dn": np.ascontiguousarray(d_slice),
                    "wqT": wqT,
                    "wkT": wkT,
                    "bqc": bqc,
                    "bkc": bkc,
                }
            )
    return in_maps


def _merge(results):
    out = np.empty((B, LQ, D), dtype=np.float32)
    for b in range(B):
        r0, r1 = results[2 * b], results[2 * b + 1]
        m0 = np.asarray(r0["mx"]).T.reshape(LQ).astype(np.float64)
        m1 = np.asarray(r1["mx"]).T.reshape(LQ).astype(np.float64)
        l0 = np.asarray(r0["ls"]).T.reshape(LQ).astype(np.float64)
        l1 = np.asarray(r1["ls"]).T.reshape(LQ).astype(np.float64)
        n0 = np.asarray(r0["num"]).astype(np.float64)
        n1 = np.asarray(r1["num"]).astype(np.float64)
        m = np.maximum(m0, m1)
        a0 = np.exp(m0 - m)
        a1 = np.exp(m1 - m)
        denom = a0 * l0 + a1 * l1
        out[b] = ((a0[:, None] * n0 + a1[:, None] * n1) / denom[:, None]).astype(
            np.float32
        )
    return out


def run(inputs, trace=False, trace_kwargs=None):
    """Run the SPMD kernel; returns (output, BassKernelResults)."""
    if "nc" not in _CACHE:
        _CACHE["nc"] = build_nc()
    nc = _CACHE["nc"]
    in_maps = _prep_inputs(**inputs)
    kw = {}
    if trace:
        kw["trace"] = True
        kw.update(trace_kwargs or {})
    res = run_bass_kernel_spmd(nc, in_maps, core_ids=list(range(8)), **kw)
    return _merge(res.results), res


def kernel(**inputs) -> np.ndarray:
    out, _ = run(inputs)
    return out


# revision 5
# speedup vs baseline: 1.1622x; 1.1622x over previous
"""Trainium2 Bass kernel for nn_AttentionMechanism (B=4, LQ=1024, ND=4096, D=1024).

Sharding: batch (4) x num_docs (2) -> 8 cores. Core c handles batch c//2 and
doc half c%2 (2048 docs). Each core computes a partial softmax-attention:
  kT = (Wk @ docsT + bk)          [e, n]   (fp32r matmuls, e on partitions)
  qT = (Wq @ queryT + bq)         [e, lq]
  s  = qT.T @ kT                  [lq, n]  per 128-row chunk, PSUM
  m  = rowmax(s); p = exp(s - m); l = rowsum(p)
  num = p @ docs                  [lq, d]
Host merges the two doc-halves per batch with the standard softmax-stat
rescale and divides by l.

All heavy matmuls run as float32r (TF32-like, full PE rate).
"""

import sys

if "/opt/trn_rl_repo" not in sys.path:
    sys.path.insert(0, "/opt/trn_rl_repo")

import numpy as np

import concourse.bass as bass  # noqa: F401
import concourse.mybir as mybir
from concourse import bacc
from concourse.tile import TileContext
from concourse.masks import make_identity
from concourse.bass_utils import run_bass_kernel_spmd

P = 128
B, LQ, ND, D = 4, 1024, 4096, 1024
N2 = ND // 2  # docs per core
EC = D // P  # 8 e-chunks
DC = D // P  # 8 d-chunks
LC = LQ // P  # 8 lq-chunks
NC = N2 // P  # 16 n-chunks
NT = N2 // 512  # 4 n-tiles of 512

F32 = mybir.dt.float32
F32R = mybir.dt.float32r
ACT = mybir.ActivationFunctionType
AX = mybir.AxisListType

_CACHE = {}


def build_nc():
    nc = bacc.Bacc("TRN2", target_bir_lowering=False)

    qT = nc.dram_tensor("qT", [D, LQ], F32, kind="ExternalInput")
    dT = nc.dram_tensor("dT", [D, N2], F32, kind="ExternalInput")
    dn = nc.dram_tensor("dn", [N2, D], F32, kind="ExternalInput")
    wqT = nc.dram_tensor("wqT", [D, D], F32, kind="ExternalInput")
    wkT = nc.dram_tensor("wkT", [D, D], F32, kind="ExternalInput")
    bqc = nc.dram_tensor("bqc", [P, EC], F32, kind="ExternalInput")
    bkc = nc.dram_tensor("bkc", [P, EC], F32, kind="ExternalInput")

    num = nc.dram_tensor("num", [LQ, D], F32, kind="ExternalOutput")
    mx = nc.dram_tensor("mx", [P, LC], F32, kind="ExternalOutput")
    ls = nc.dram_tensor("ls", [P, LC], F32, kind="ExternalOutput")

    qT_r = qT.ap().rearrange("(dc p) l -> p dc l", p=P).bitcast(F32R)
    dT_r = dT.ap().rearrange("(dc p) n -> p dc n", p=P).bitcast(F32R)
    dn_r = dn.ap().rearrange("(nc p) d -> p nc d", p=P).bitcast(F32R)
    wqT_r = wqT.ap().rearrange("(dc p) e -> p dc e", p=P).bitcast(F32R)
    wkT_r = wkT.ap().rearrange("(dc p) e -> p dc e", p=P).bitcast(F32R)

    with TileContext(nc) as tc:
        with (
            tc.tile_pool(name="const", bufs=1) as cpool,
            tc.tile_pool(name="stats", bufs=1) as spool,
            tc.tile_pool(name="kTp", bufs=1) as kT_pool,
        ):
            ident32 = cpool.tile([P, P], F32)
            make_identity(nc, ident32[:])
            ident = cpool.tile([P, P], F32R)
            nc.vector.tensor_copy(ident[:], ident32[:])
            bq_s = cpool.tile([P, EC], F32)
            bk_s = cpool.tile([P, EC], F32)
            nc.sync.dma_start(bq_s[:], bqc.ap())
            nc.sync.dma_start(bk_s[:], bkc.ap())

            mx_all = spool.tile([P, LC], F32)
            ls_all = spool.tile([P, LC], F32)

            kT = [kT_pool.tile([P, N2], F32R, name=f"kT{ec}") for ec in range(EC)]

            # ---- Phase K: kT[e, n] = Wk @ docsT + bk ----
            with (
                tc.tile_pool(name="pk", bufs=1) as pk,
                tc.tile_pool(name="psk", bufs=8, space="PSUM") as psk,
            ):
                wk_t, dT_t = [], []
                for dc in range(DC):
                    w = pk.tile([P, D], F32R, name=f"wk{dc}")
                    d = pk.tile([P, N2], F32R, name=f"dTt{dc}")
                    nc.sync.dma_start(w[:], wkT_r[:, dc, :])
                    nc.sync.dma_start(d[:], dT_r[:, dc, :])
                    wk_t.append(w)
                    dT_t.append(d)
                for ec in range(EC):
                    pss = [psk.tile([P, 512], F32, name="psk") for t in range(NT)]
                    for dc in range(DC):
                        for t in range(NT):
                            nc.tensor.matmul(
                                pss[t][:],
                                wk_t[dc][:, ec * P : (ec + 1) * P],
                                dT_t[dc][:, t * 512 : (t + 1) * 512],
                                start=(dc == 0),
                                stop=(dc == DC - 1),
                            )
                    for t in range(NT):
                        nc.scalar.activation(
                            kT[ec][:, t * 512 : (t + 1) * 512],
                            pss[t][:],
                            ACT.Identity,
                            bias=bk_s[:, ec : ec + 1],
                        )

            with tc.tile_pool(name="qTp", bufs=1) as qTp_pool:
                qTp = [
                    qTp_pool.tile([P, LQ], F32R, name=f"qTp{ec}") for ec in range(EC)
                ]

                # ---- Phase Q: qTp[e, lq] = Wq @ queryT + bq ----
                with (
                    tc.tile_pool(name="pq", bufs=1) as pq,
                    tc.tile_pool(name="psq", bufs=4, space="PSUM") as psq,
                ):
                    wq_t, qT_t = [], []
                    for dc in range(DC):
                        w = pq.tile([P, D], F32R, name=f"wq{dc}")
                        q = pq.tile([P, LQ], F32R, name=f"qTt{dc}")
                        nc.sync.dma_start(w[:], wqT_r[:, dc, :])
                        nc.sync.dma_start(q[:], qT_r[:, dc, :])
                        wq_t.append(w)
                        qT_t.append(q)
                    for ec in range(EC):
                        pss = [psq.tile([P, 512], F32, name="psq") for t in range(2)]
                        for dc in range(DC):
                            for t in range(2):
                                nc.tensor.matmul(
                                    pss[t][:],
                                    wq_t[dc][:, ec * P : (ec + 1) * P],
                                    qT_t[dc][:, t * 512 : (t + 1) * 512],
                                    start=(dc == 0),
                                    stop=(dc == DC - 1),
                                )
                        for t in range(2):
                            nc.scalar.activation(
                                qTp[ec][:, t * 512 : (t + 1) * 512],
                                pss[t][:],
                                ACT.Identity,
                                bias=bq_s[:, ec : ec + 1],
                            )

                # ---- Phase A: attention per 128-query chunk ----
                with (
                    tc.tile_pool(name="pa", bufs=1) as pa,
                    tc.tile_pool(name="pwork", bufs=2) as pw,
                    tc.tile_pool(name="ps_sc", bufs=1, space="PSUM") as ps_sc,
                    tc.tile_pool(name="ps_av", bufs=1, space="PSUM") as ps_av,
                    tc.tile_pool(name="ps_tp", bufs=2, space="PSUM") as ps_tp,
                ):
                    # dn loads on SWDGE (gpsimd) queues: keeps the sync queue
                    # free so phase-A PE work isn't gated behind this drain.
                    dn_s = []
                    for i in range(NC):
                        t = pa.tile([P, D], F32R, name=f"dn{i}")
                        nc.gpsimd.dma_start(t[:], dn_r[:, i, :])
                        dn_s.append(t)

                    for lc in range(LC):
                        lq_sl = slice(lc * P, (lc + 1) * P)
                        # scores, one 512-wide PSUM tile at a time
                        sc_t = []
                        mx4 = pw.tile([P, NT], F32, name="mx4")
                        ls4 = pw.tile([P, NT], F32, name="ls4")
                        for t in range(NT):
                            sc = ps_sc.tile([P, 512], F32, name=f"sc{t}")
                            for ec in range(EC):
                                nc.tensor.matmul(
                                    sc[:],
                                    qTp[ec][:, lq_sl],
                                    kT[ec][:, t * 512 : (t + 1) * 512],
                                    start=(ec == 0),
                                    stop=(ec == EC - 1),
                                )
                            # rowmax of this tile overlaps the next tile's matmuls
                            nc.vector.reduce_max(mx4[:, t : t + 1], sc[:], axis=AX.X)
                            sc_t.append(sc)
                        negmax = pw.tile([P, 1], F32, name="negmax")
                        nc.vector.reduce_max(
                            negmax[:], mx4[:], axis=AX.X, negate=True
                        )
                        nc.vector.tensor_scalar_mul(
                            mx_all[:, lc : lc + 1], negmax[:], -1.0
                        )
                        # per 512-group: exp -> transpose -> AV, interleaved
                        av = ps_av.tile([P, D], F32, name="av")
                        for g in range(NT):
                            probs = pw.tile([P, 512], F32R, name=f"probs{g}")
                            nc.scalar.activation(
                                probs[:],
                                sc_t[g][:],
                                ACT.Exp,
                                bias=negmax[:],
                                accum_out=ls4[:, g : g + 1],
                            )
                            tp = ps_tp.tile([P, 512], F32R, name="tp")
                            for j in range(4):
                                nc.tensor.transpose(
                                    tp[:, j * P : (j + 1) * P],
                                    probs[:, j * P : (j + 1) * P],
                                    ident[:],
                                )
                            probsT = pw.tile([P, 4, P], F32R, name=f"probsT{g}")
                            nc.vector.tensor_copy(probsT[:], tp[:])
                            for j in range(4):
                                nn = g * 4 + j
                                for dh in range(2):
                                    nc.tensor.matmul(
                                        av[:, dh * 512 : (dh + 1) * 512],
                                        probsT[:, j, :],
                                        dn_s[nn][:, dh * 512 : (dh + 1) * 512],
                                        start=(nn == 0),
                                        stop=(nn == NC - 1),
                                    )
                        nc.vector.reduce_sum(
                            ls_all[:, lc : lc + 1], ls4[:], axis=AX.X
                        )
                        num_t = pw.tile([P, D], F32, name="num_t")
                        nc.scalar.activation(num_t[:], av[:], ACT.Copy)
                        nc.sync.dma_start(num.ap()[lq_sl, :], num_t[:])

            nc.sync.dma_start(mx.ap()[:, :], mx_all[:])
            nc.sync.dma_start(ls.ap()[:, :], ls_all[:])

    nc.compile()
    return nc


def _prep_inputs(query, documents, Wq, bq, Wk, bk):
    query = np.asarray(query, dtype=np.float32)
    documents = np.asarray(documents, dtype=np.float32)
    wqT = np.ascontiguousarray(np.asarray(Wq, np.float32).T)
    wkT = np.ascontiguousarray(np.asarray(Wk, np.float32).T)
    bqc = np.ascontiguousarray(np.asarray(bq, np.float32).reshape(EC, P).T)
    bkc = np.ascontiguousarray(np.asarray(bk, np.float32).reshape(EC, P).T)
    in_maps = []
    for b in range(B):
        qTh = np.ascontiguousarray(query[b].T)
        for h in range(2):
            d_slice = documents[b, h * N2 : (h + 1) * N2]
            in_maps.append(
                {
                    "qT": qTh,
                    "dT": np.ascontiguousarray(d_slice.T),
                    "dn": np.ascontiguousarray(d_slice),
                    "wqT": wqT,
                    "wkT": wkT,
                    "bqc": bqc,
                    "bkc": bkc,
                }
            )
    return in_maps


def _merge(results):
    out = np.empty((B, LQ, D), dtype=np.float32)
    for b in range(B):
        r0, r1 = results[2 * b], results[2 * b + 1]
        m0 = np.asarray(r0["mx"]).T.reshape(LQ).astype(np.float64)
        m1 = np.asarray(r1["mx"]).T.reshape(LQ).astype(np.float64)
        l0 = np.asarray(r0["ls"]).T.reshape(LQ).astype(np.float64)
        l1 = np.asarray(r1["ls"]).T.reshape(LQ).astype(np.float64)
        n0 = np.asarray(r0["num"]).astype(np.float64)
        n1 = np.asarray(r1["num"]).astype(np.float64)
        m = np.maximum(m0, m1)
        a0 = np.exp(m0 - m)
        a1 = np.exp(m1 - m)
        denom = a0 * l0 + a1 * l1
        out[b] = ((a0[:, None] * n0 + a1[:, None] * n1) / denom[:, None]).astype(
            np.float32
        )
    return out


def run(inputs, trace=False, trace_kwargs=None):
    """Run the SPMD kernel; returns (output, BassKernelResults)."""
    if "nc" not in _CACHE:
        _CACHE["nc"] = build_nc()
    nc = _CACHE["nc"]
    in_maps = _prep_inputs(**inputs)
    kw = {}
    if trace:
        kw["trace"] = True
        kw.update(trace_kwargs or {})
    res = run_bass_kernel_spmd(nc, in_maps, core_ids=list(range(8)), **kw)
    return _merge(res.results), res


def kernel(**inputs) -> np.ndarray:
    out, _ = run(inputs)
    return out


# revision 6
# speedup vs baseline: 1.2167x; 1.0469x over previous
"""Trainium2 Bass kernel for nn_AttentionMechanism (B=4, LQ=1024, ND=4096, D=1024).

Sharding: batch (4) x num_docs (2) -> 8 cores. Core c handles batch c//2 and
doc half c%2 (2048 docs). Each core computes a partial softmax-attention:
  kT = (Wk @ docsT + bk)          [e, n]   (fp32r matmuls, e on partitions)
  qT = (Wq @ queryT + bq)         [e, lq]
  s  = qT.T @ kT                  [lq, n]  per 128-row chunk, PSUM
  m  = rowmax(s); p = exp(s - m); l = rowsum(p)
  num = p @ docs                  [lq, d]
Host merges the two doc-halves per batch with the standard softmax-stat
rescale and divides by l.

All heavy matmuls run as float32r (TF32-like, full PE rate).
"""

import sys

if "/opt/trn_rl_repo" not in sys.path:
    sys.path.insert(0, "/opt/trn_rl_repo")

import numpy as np

import concourse.bass as bass  # noqa: F401
import concourse.mybir as mybir
from concourse import bacc
from concourse.tile import TileContext
from concourse.masks import make_identity
from concourse.bass_utils import run_bass_kernel_spmd

P = 128
B, LQ, ND, D = 4, 1024, 4096, 1024
N2 = ND // 2  # docs per core
EC = D // P  # 8 e-chunks
DC = D // P  # 8 d-chunks
LC = LQ // P  # 8 lq-chunks
NC = N2 // P  # 16 n-chunks
NT = N2 // 512  # 4 n-tiles of 512

F32 = mybir.dt.float32
F32R = mybir.dt.float32r
ACT = mybir.ActivationFunctionType
AX = mybir.AxisListType

_CACHE = {}


def build_nc():
    nc = bacc.Bacc("TRN2", target_bir_lowering=False)

    qT = nc.dram_tensor("qT", [D, LQ], F32, kind="ExternalInput")
    dT = nc.dram_tensor("dT", [D, N2], F32, kind="ExternalInput")
    dn = nc.dram_tensor("dn", [N2, D], F32, kind="ExternalInput")
    wqT = nc.dram_tensor("wqT", [D, D], F32, kind="ExternalInput")
    wkT = nc.dram_tensor("wkT", [D, D], F32, kind="ExternalInput")
    bqc = nc.dram_tensor("bqc", [P, EC], F32, kind="ExternalInput")
    bkc = nc.dram_tensor("bkc", [P, EC], F32, kind="ExternalInput")

    num = nc.dram_tensor("num", [LQ, D], F32, kind="ExternalOutput")
    mx = nc.dram_tensor("mx", [P, LC], F32, kind="ExternalOutput")
    ls = nc.dram_tensor("ls", [P, LC], F32, kind="ExternalOutput")

    qT_r = qT.ap().rearrange("(dc p) l -> p dc l", p=P).bitcast(F32R)
    dT_r = dT.ap().rearrange("(dc p) n -> p dc n", p=P).bitcast(F32R)
    dn_r = dn.ap().rearrange("(nc p) d -> p nc d", p=P).bitcast(F32R)
    wqT_r = wqT.ap().rearrange("(dc p) e -> p dc e", p=P).bitcast(F32R)
    wkT_r = wkT.ap().rearrange("(dc p) e -> p dc e", p=P).bitcast(F32R)

    with TileContext(nc) as tc:
        with (
            tc.tile_pool(name="const", bufs=1) as cpool,
            tc.tile_pool(name="stats", bufs=1) as spool,
            tc.tile_pool(name="kTp", bufs=1) as kT_pool,
        ):
            ident32 = cpool.tile([P, P], F32)
            make_identity(nc, ident32[:])
            ident = cpool.tile([P, P], F32R)
            nc.vector.tensor_copy(ident[:], ident32[:])
            bq_s = cpool.tile([P, EC], F32)
            bk_s = cpool.tile([P, EC], F32)
            nc.sync.dma_start(bq_s[:], bqc.ap())
            nc.sync.dma_start(bk_s[:], bkc.ap())

            mx_all = spool.tile([P, LC], F32)
            ls_all = spool.tile([P, LC], F32)

            kT = [kT_pool.tile([P, N2], F32R, name=f"kT{ec}") for ec in range(EC)]

            # ---- Phase K: kT[e, n] = Wk @ docsT + bk ----
            with (
                tc.tile_pool(name="pk", bufs=1) as pk,
                tc.tile_pool(name="psk", bufs=8, space="PSUM") as psk,
            ):
                wk_t, dT_t = [], []
                for dc in range(DC):
                    w = pk.tile([P, D], F32R, name=f"wk{dc}")
                    d = pk.tile([P, N2], F32R, name=f"dTt{dc}")
                    nc.sync.dma_start(w[:], wkT_r[:, dc, :])
                    nc.sync.dma_start(d[:], dT_r[:, dc, :])
                    wk_t.append(w)
                    dT_t.append(d)
                for ec in range(EC):
                    pss = [psk.tile([P, 512], F32, name="psk") for t in range(NT)]
                    for dc in range(DC):
                        for t in range(NT):
                            nc.tensor.matmul(
                                pss[t][:],
                                wk_t[dc][:, ec * P : (ec + 1) * P],
                                dT_t[dc][:, t * 512 : (t + 1) * 512],
                                start=(dc == 0),
                                stop=(dc == DC - 1),
                            )
                    for t in range(NT):
                        nc.scalar.activation(
                            kT[ec][:, t * 512 : (t + 1) * 512],
                            pss[t][:],
                            ACT.Identity,
                            bias=bk_s[:, ec : ec + 1],
                        )

            with tc.tile_pool(name="qTp", bufs=1) as qTp_pool:
                qTp = [
                    qTp_pool.tile([P, LQ], F32R, name=f"qTp{ec}") for ec in range(EC)
                ]

                # ---- Phase Q: qTp[e, lq] = Wq @ queryT + bq ----
                with (
                    tc.tile_pool(name="pq", bufs=1) as pq,
                    tc.tile_pool(name="psq", bufs=4, space="PSUM") as psq,
                ):
                    wq_t, qT_t = [], []
                    for dc in range(DC):
                        w = pq.tile([P, D], F32R, name=f"wq{dc}")
                        q = pq.tile([P, LQ], F32R, name=f"qTt{dc}")
                        nc.sync.dma_start(w[:], wqT_r[:, dc, :])
                        nc.sync.dma_start(q[:], qT_r[:, dc, :])
                        wq_t.append(w)
                        qT_t.append(q)
                    for ec in range(EC):
                        pss = [psq.tile([P, 512], F32, name="psq") for t in range(2)]
                        for dc in range(DC):
                            for t in range(2):
                                nc.tensor.matmul(
                                    pss[t][:],
                                    wq_t[dc][:, ec * P : (ec + 1) * P],
                                    qT_t[dc][:, t * 512 : (t + 1) * 512],
                                    start=(dc == 0),
                                    stop=(dc == DC - 1),
                                )
                        for t in range(2):
                            nc.scalar.activation(
                                qTp[ec][:, t * 512 : (t + 1) * 512],
                                pss[t][:],
                                ACT.Identity,
                                bias=bq_s[:, ec : ec + 1],
                            )

                # ---- Phase A: attention per 128-query chunk ----
                with (
                    tc.tile_pool(name="pa", bufs=1) as pa,
                    tc.tile_pool(name="pwork", bufs=2) as pw,
                    tc.tile_pool(name="ps_sc", bufs=5, space="PSUM") as ps_sc,
                    tc.tile_pool(name="ps_av", bufs=1, space="PSUM") as ps_av,
                    tc.tile_pool(name="ps_tp", bufs=1, space="PSUM") as ps_tp,
                ):
                    # dn loads on SWDGE (gpsimd) queues: keeps the sync queue
                    # free so phase-A PE work isn't gated behind this drain.
                    dn_s = []
                    for i in range(NC):
                        t = pa.tile([P, D], F32R, name=f"dn{i}")
                        nc.gpsimd.dma_start(t[:], dn_r[:, i, :])
                        dn_s.append(t)

                    # Software pipeline: the next chunk's score matmuls are
                    # emitted into the softmax-latency stall of the current
                    # chunk, using a 5-slot rotating score-PSUM pool.
                    scs = {}
                    mx4s = {}

                    def emit_scores(lc, ts):
                        lq_sl = slice(lc * P, (lc + 1) * P)
                        if lc not in mx4s:
                            mx4s[lc] = pw.tile([P, NT], F32, name="mx4")
                        for ec in range(EC):
                            for t in ts:
                                if (lc, t) not in scs:
                                    scs[(lc, t)] = ps_sc.tile(
                                        [P, 512], F32, name="sc"
                                    )
                                nc.tensor.matmul(
                                    scs[(lc, t)][:],
                                    qTp[ec][:, lq_sl],
                                    kT[ec][:, t * 512 : (t + 1) * 512],
                                    start=(ec == 0),
                                    stop=(ec == EC - 1),
                                )
                        for t in ts:
                            nc.vector.reduce_max(
                                mx4s[lc][:, t : t + 1], scs[(lc, t)][:], axis=AX.X
                            )

                    emit_scores(0, [0, 1])
                    emit_scores(0, [2, 3])
                    for lc in range(LC):
                        lq_sl = slice(lc * P, (lc + 1) * P)
                        mx4 = mx4s.pop(lc)
                        ls4 = pw.tile([P, NT], F32, name="ls4")
                        negmax = pw.tile([P, 1], F32, name="negmax")
                        nc.vector.reduce_max(
                            negmax[:], mx4[:], axis=AX.X, negate=True
                        )
                        nc.vector.tensor_scalar_mul(
                            mx_all[:, lc : lc + 1], negmax[:], -1.0
                        )
                        if lc + 1 < LC:
                            emit_scores(lc + 1, [0, 1])
                        # per 512-group: exp -> transpose -> AV, interleaved
                        av = ps_av.tile([P, D], F32, name="av")
                        for g in range(NT):
                            sc = scs.pop((lc, g))
                            probs = pw.tile([P, 512], F32R, name=f"probs{g}")
                            nc.scalar.activation(
                                probs[:],
                                sc[:],
                                ACT.Exp,
                                bias=negmax[:],
                                accum_out=ls4[:, g : g + 1],
                            )
                            tp = ps_tp.tile([P, 512], F32R, name="tp")
                            for j in range(4):
                                nc.tensor.transpose(
                                    tp[:, j * P : (j + 1) * P],
                                    probs[:, j * P : (j + 1) * P],
                                    ident[:],
                                )
                            probsT = pw.tile([P, 4, P], F32R, name=f"probsT{g}")
                            nc.vector.tensor_copy(probsT[:], tp[:])
                            for j in range(4):
                                nn = g * 4 + j
                                for dh in range(2):
                                    nc.tensor.matmul(
                                        av[:, dh * 512 : (dh + 1) * 512],
                                        probsT[:, j, :],
                                        dn_s[nn][:, dh * 512 : (dh + 1) * 512],
                                        start=(nn == 0),
                                        stop=(nn == NC - 1),
                                    )
                            if g == 0 and lc + 1 < LC:
                                emit_scores(lc + 1, [2, 3])
                        nc.vector.reduce_sum(
                            ls_all[:, lc : lc + 1], ls4[:], axis=AX.X
                        )
                        num_t = pw.tile([P, D], F32, name="num_t")
                        nc.scalar.activation(num_t[:], av[:], ACT.Copy)
                        nc.sync.dma_start(num.ap()[lq_sl, :], num_t[:])

            nc.sync.dma_start(mx.ap()[:, :], mx_all[:])
            nc.sync.dma_start(ls.ap()[:, :], ls_all[:])

    nc.compile()
    return nc


def _prep_inputs(query, documents, Wq, bq, Wk, bk):
    query = np.asarray(query, dtype=np.float32)
    documents = np.asarray(documents, dtype=np.float32)
    wqT = np.ascontiguousarray(np.asarray(Wq, np.float32).T)
    wkT = np.ascontiguousarray(np.asarray(Wk, np.float32).T)
    bqc = np.ascontiguousarray(np.asarray(bq, np.float32).reshape(EC, P).T)
    bkc = np.ascontiguousarray(np.asarray(bk, np.float32).reshape(EC, P).T)
    in_maps = []
    for b in range(B):
        qTh = np.ascontiguousarray(query[b].T)
        for h in range(2):
            d_slice = documents[b, h * N2 : (h + 1) * N2]
            in_maps.append(
                {
                    "qT": qTh,
                    "dT": np.ascontiguousarray(d_slice.T),
                    "dn": np.ascontiguousarray(d_slice),
                    "wqT": wqT,
                    "wkT": wkT,
                    "bqc": bqc,
                    "bkc": bkc,
                }
            )
    return in_maps


def _merge(results):
    out = np.empty((B, LQ, D), dtype=np.float32)
    for b in range(B):
        r0, r1 = results[2 * b], results[2 * b + 1]
        m0 = np.asarray(r0["mx"]).T.reshape(LQ).astype(np.float64)
        m1 = np.asarray(r1["mx"]).T.reshape(LQ).astype(np.float64)
        l0 = np.asarray(r0["ls"]).T.reshape(LQ).astype(np.float64)
        l1 = np.asarray(r1["ls"]).T.reshape(LQ).astype(np.float64)
        n0 = np.asarray(r0["num"]).astype(np.float64)
        n1 = np.asarray(r1["num"]).astype(np.float64)
        m = np.maximum(m0, m1)
        a0 = np.exp(m0 - m)
        a1 = np.exp(m1 - m)
        denom = a0 * l0 + a1 * l1
        out[b] = ((a0[:, None] * n0 + a1[:, None] * n1) / denom[:, None]).astype(
            np.float32
        )
    return out


def run(inputs, trace=False, trace_kwargs=None):
    """Run the SPMD kernel; returns (output, BassKernelResults)."""
    if "nc" not in _CACHE:
        _CACHE["nc"] = build_nc()
    nc = _CACHE["nc"]
    in_maps = _prep_inputs(**inputs)
    kw = {}
    if trace:
        kw["trace"] = True
        kw.update(trace_kwargs or {})
    res = run_bass_kernel_spmd(nc, in_maps, core_ids=list(range(8)), **kw)
    return _merge(res.results), res


def kernel(**inputs) -> np.ndarray:
    out, _ = run(inputs)
    return out


# revision 8
# speedup vs baseline: 1.2935x; 1.0631x over previous
"""Trainium2 Bass kernel for nn_AttentionMechanism (B=4, LQ=1024, ND=4096, D=1024).

Sharding: batch (4) x num_docs (2) -> 8 cores. Core c handles batch c//2 and
doc half c%2 (2048 docs). Each core computes a partial softmax-attention:
  kT = (Wk @ docsT + bk)          [e, n]   (fp32r matmuls, e on partitions)
  qT = (Wq @ queryT + bq)         [e, lq]
  s  = qT.T @ kT                  [lq, n]  per 128-row chunk, PSUM
  m  = rowmax(s); p = exp(s - m); l = rowsum(p)
  num = p @ docs                  [lq, d]
Host merges the two doc-halves per batch with the standard softmax-stat
rescale and divides by l.

All heavy matmuls run as float32r (TF32-like, full PE rate).
"""

import sys

if "/opt/trn_rl_repo" not in sys.path:
    sys.path.insert(0, "/opt/trn_rl_repo")

import numpy as np

import concourse.bass as bass  # noqa: F401
import concourse.mybir as mybir
from concourse import bacc
from concourse.tile import TileContext
from concourse.masks import make_identity
from concourse.bass_utils import run_bass_kernel_spmd

P = 128
B, LQ, ND, D = 4, 1024, 4096, 1024
N2 = ND // 2  # docs per core
EC = D // P  # 8 e-chunks
DC = D // P  # 8 d-chunks
LC = LQ // P  # 8 lq-chunks
NC = N2 // P  # 16 n-chunks
NT = N2 // 512  # 4 n-tiles of 512

F32 = mybir.dt.float32
F32R = mybir.dt.float32r
ACT = mybir.ActivationFunctionType
AX = mybir.AxisListType

_CACHE = {}


def build_nc():
    nc = bacc.Bacc("TRN2", target_bir_lowering=False)

    qT = nc.dram_tensor("qT", [D, LQ], F32, kind="ExternalInput")
    dT = nc.dram_tensor("dT", [D, N2], F32, kind="ExternalInput")
    dn = nc.dram_tensor("dn", [N2, D], F32, kind="ExternalInput")
    wqT = nc.dram_tensor("wqT", [D, D], F32, kind="ExternalInput")
    wkT = nc.dram_tensor("wkT", [D, D], F32, kind="ExternalInput")
    bqc = nc.dram_tensor("bqc", [P, EC], F32, kind="ExternalInput")
    bkc = nc.dram_tensor("bkc", [P, EC], F32, kind="ExternalInput")

    num = nc.dram_tensor("num", [LQ, D], F32, kind="ExternalOutput")
    mx = nc.dram_tensor("mx", [P, LC], F32, kind="ExternalOutput")
    ls = nc.dram_tensor("ls", [P, LC], F32, kind="ExternalOutput")

    qT_r = qT.ap().rearrange("(dc p) l -> p dc l", p=P).bitcast(F32R)
    dT_r = dT.ap().rearrange("(dc p) n -> p dc n", p=P).bitcast(F32R)
    dn_r = dn.ap().rearrange("(nc p) d -> p nc d", p=P).bitcast(F32R)
    wqT_r = wqT.ap().rearrange("(dc p) e -> p dc e", p=P).bitcast(F32R)
    wkT_r = wkT.ap().rearrange("(dc p) e -> p dc e", p=P).bitcast(F32R)

    with TileContext(nc) as tc:
        with (
            tc.tile_pool(name="const", bufs=1) as cpool,
            tc.tile_pool(name="stats", bufs=1) as spool,
            tc.tile_pool(name="kTp", bufs=1) as kT_pool,
        ):
            ident32 = cpool.tile([P, P], F32)
            make_identity(nc, ident32[:])
            ident = cpool.tile([P, P], F32R)
            nc.vector.tensor_copy(ident[:], ident32[:])
            bq_s = cpool.tile([P, EC], F32)
            bk_s = cpool.tile([P, EC], F32)
            nc.sync.dma_start(bq_s[:], bqc.ap())
            nc.sync.dma_start(bk_s[:], bkc.ap())

            mx_all = spool.tile([P, LC], F32)
            ls_all = spool.tile([P, LC], F32)

            kT = [kT_pool.tile([P, N2], F32R, name=f"kT{ec}") for ec in range(EC)]

            with tc.tile_pool(name="qTp", bufs=1) as qTp_pool:
                qTp = [
                    qTp_pool.tile([P, LQ], F32R, name=f"qTp{ec}") for ec in range(EC)
                ]

                # ---- Phase Q: qTp[e, lq] = Wq @ queryT + bq ----
                pk_pre_ctx = tc.tile_pool(name="pk_pre", bufs=1)
                pk_pre = pk_pre_ctx.__enter__()
                with (
                    tc.tile_pool(name="pq", bufs=1) as pq,
                    tc.tile_pool(name="psq", bufs=4, space="PSUM") as psq,
                ):
                    wq_t, qT_t = [], []
                    for dc in range(DC):
                        w = pq.tile([P, D], F32R, name=f"wq{dc}")
                        q = pq.tile([P, LQ], F32R, name=f"qTt{dc}")
                        nc.sync.dma_start(w[:], wqT_r[:, dc, :])
                        nc.sync.dma_start(q[:], qT_r[:, dc, :])
                        wq_t.append(w)
                        qT_t.append(q)
                    # prefetch the first two K-phase input chunks behind the
                    # Q inputs on the DMA queues, into a pool that survives Q
                    wk_t, dT_t = [], []
                    for dc in range(2):
                        w = pk_pre.tile([P, D], F32R, name=f"wk{dc}")
                        d = pk_pre.tile([P, N2], F32R, name=f"dTt{dc}")
                        nc.sync.dma_start(w[:], wkT_r[:, dc, :])
                        nc.sync.dma_start(d[:], dT_r[:, dc, :])
                        wk_t.append(w)
                        dT_t.append(d)
                    for ec in range(EC):
                        pss = [psq.tile([P, 512], F32, name="psq") for t in range(2)]
                        for dc in range(DC):
                            for t in range(2):
                                nc.tensor.matmul(
                                    pss[t][:],
                                    wq_t[dc][:, ec * P : (ec + 1) * P],
                                    qT_t[dc][:, t * 512 : (t + 1) * 512],
                                    start=(dc == 0),
                                    stop=(dc == DC - 1),
                                )
                        for t in range(2):
                            nc.scalar.activation(
                                qTp[ec][:, t * 512 : (t + 1) * 512],
                                pss[t][:],
                                ACT.Identity,
                                bias=bq_s[:, ec : ec + 1],
                            )

                # ---- Phase K: kT[e, n] = Wk @ docsT + bk ----
                with (
                    tc.tile_pool(name="pk", bufs=1) as pk,
                    tc.tile_pool(name="psk", bufs=8, space="PSUM") as psk,
                ):
                    for dc in range(2, DC):
                        w = pk.tile([P, D], F32R, name=f"wk{dc}")
                        d = pk.tile([P, N2], F32R, name=f"dTt{dc}")
                        nc.sync.dma_start(w[:], wkT_r[:, dc, :])
                        nc.sync.dma_start(d[:], dT_r[:, dc, :])
                        wk_t.append(w)
                        dT_t.append(d)
                    for ec in range(EC):
                        pss = [psk.tile([P, 512], F32, name="psk") for t in range(NT)]
                        for dc in range(DC):
                            for t in range(NT):
                                nc.tensor.matmul(
                                    pss[t][:],
                                    wk_t[dc][:, ec * P : (ec + 1) * P],
                                    dT_t[dc][:, t * 512 : (t + 1) * 512],
                                    start=(dc == 0),
                                    stop=(dc == DC - 1),
                                )
                        for t in range(NT):
                            nc.scalar.activation(
                                kT[ec][:, t * 512 : (t + 1) * 512],
                                pss[t][:],
                                ACT.Identity,
                                bias=bk_s[:, ec : ec + 1],
                            )

                pk_pre_ctx.__exit__(None, None, None)

                # ---- Phase A: attention per 128-query chunk ----
                with (
                    tc.tile_pool(name="pa", bufs=1) as pa,
                    tc.tile_pool(name="pwork", bufs=2) as pw,
                    tc.tile_pool(name="ps_sc", bufs=5, space="PSUM") as ps_sc,
                    tc.tile_pool(name="ps_av", bufs=1, space="PSUM") as ps_av,
                    tc.tile_pool(name="ps_tp", bufs=1, space="PSUM") as ps_tp,
                ):
                    # dn loads on SWDGE (gpsimd) queues: keeps the sync queue
                    # free so phase-A PE work isn't gated behind this drain.
                    dn_s = []
                    for i in range(NC):
                        t = pa.tile([P, D], F32R, name=f"dn{i}")
                        nc.gpsimd.dma_start(t[:], dn_r[:, i, :])
                        dn_s.append(t)

                    # Software pipeline: the next chunk's score matmuls are
                    # emitted into the softmax-latency stall of the current
                    # chunk, using a 5-slot rotating score-PSUM pool.
                    scs = {}
                    mx4s = {}

                    def emit_scores(lc, ts):
                        lq_sl = slice(lc * P, (lc + 1) * P)
                        if lc not in mx4s:
                            mx4s[lc] = pw.tile([P, NT], F32, name="mx4")
                        for ec in range(EC):
                            for t in ts:
                                if (lc, t) not in scs:
                                    scs[(lc, t)] = ps_sc.tile(
                                        [P, 512], F32, name="sc"
                                    )
                                nc.tensor.matmul(
                                    scs[(lc, t)][:],
                                    qTp[ec][:, lq_sl],
                                    kT[ec][:, t * 512 : (t + 1) * 512],
                                    start=(ec == 0),
                                    stop=(ec == EC - 1),
                                )
                        for t in ts:
                            nc.vector.reduce_max(
                                mx4s[lc][:, t : t + 1], scs[(lc, t)][:], axis=AX.X
                            )

                    emit_scores(0, [0, 1])
                    emit_scores(0, [2, 3])
                    for lc in range(LC):
                        lq_sl = slice(lc * P, (lc + 1) * P)
                        mx4 = mx4s.pop(lc)
                        ls8 = pw.tile([P, 2 * NT], F32, name="ls8")
                        negmax = pw.tile([P, 1], F32, name="negmax")
                        nc.vector.reduce_max(
                            negmax[:], mx4[:], axis=AX.X, negate=True
                        )
                        nc.vector.tensor_scalar_mul(
                            mx_all[:, lc : lc + 1], negmax[:], -1.0
                        )
                        if lc + 1 < LC:
                            emit_scores(lc + 1, [0, 1])
                        # per 512-group: exp -> transpose -> AV, interleaved
                        av = ps_av.tile([P, D], F32, name="av")
                        for g in range(NT):
                            sc = scs.pop((lc, g))
                            probs_h = [
                                pw.tile([P, 256], F32R, name=f"probs{g}_{h}")
                                for h in range(2)
                            ]
                            for h in range(2):
                                nc.scalar.activation(
                                    probs_h[h][:],
                                    sc[:, h * 256 : (h + 1) * 256],
                                    ACT.Exp,
                                    bias=negmax[:],
                                    accum_out=ls8[:, 2 * g + h : 2 * g + h + 1],
                                )
                            tp = ps_tp.tile([P, 512], F32R, name="tp")
                            for j in range(4):
                                nc.tensor.transpose(
                                    tp[:, j * P : (j + 1) * P],
                                    probs_h[j // 2][:, (j % 2) * P : (j % 2 + 1) * P],
                                    ident[:],
                                )
                            probsT = pw.tile([P, 4, P], F32R, name=f"probsT{g}")
                            nc.vector.tensor_copy(probsT[:], tp[:])
                            for j in range(4):
                                nn = g * 4 + j
                                for dh in range(2):
                                    nc.tensor.matmul(
                                        av[:, dh * 512 : (dh + 1) * 512],
                                        probsT[:, j, :],
                                        dn_s[nn][:, dh * 512 : (dh + 1) * 512],
                                        start=(nn == 0),
                                        stop=(nn == NC - 1),
                                    )
                            if g == 0 and lc + 1 < LC:
                                emit_scores(lc + 1, [2, 3])
                        nc.vector.reduce_sum(
                            ls_all[:, lc : lc + 1], ls8[:], axis=AX.X
                        )
                        num_t = pw.tile([P, D], F32, name="num_t")
                        nc.scalar.activation(num_t[:], av[:], ACT.Copy)
                        nc.sync.dma_start(num.ap()[lq_sl, :], num_t[:])

            nc.sync.dma_start(mx.ap()[:, :], mx_all[:])
            nc.sync.dma_start(ls.ap()[:, :], ls_all[:])

    nc.compile()
    return nc


def _prep_inputs(query, documents, Wq, bq, Wk, bk):
    query = np.asarray(query, dtype=np.float32)
    documents = np.asarray(documents, dtype=np.float32)
    wqT = np.ascontiguousarray(np.asarray(Wq, np.float32).T)
    wkT = np.ascontiguousarray(np.asarray(Wk, np.float32).T)
    bqc = np.ascontiguousarray(np.asarray(bq, np.float32).reshape(EC, P).T)
    bkc = np.ascontiguousarray(np.asarray(bk, np.float32).reshape(EC, P).T)
    in_maps = []
    for b in range(B):
        qTh = np.ascontiguousarray(query[b].T)
        for h in range(2):
            d_slice = documents[b, h * N2 : (h + 1) * N2]
            in_maps.append(
                {
                    "qT": qTh,
                    "dT": np.ascontiguousarray(d_slice.T),
                    "dn": np.ascontiguousarray(d_slice),
                    "wqT": wqT,
                    "wkT": wkT,
                    "bqc": bqc,
                    "bkc": bkc,
                }
            )
    return in_maps


def _merge(results):
    out = np.empty((B, LQ, D), dtype=np.float32)
    for b in range(B):
        r0, r1 = results[2 * b], results[2 * b + 1]
        m0 = np.asarray(r0["mx"]).T.reshape(LQ).astype(np.float64)
        m1 = np.asarray(r1["mx"]).T.reshape(LQ).astype(np.float64)
        l0 = np.asarray(r0["ls"]).T.reshape(LQ).astype(np.float64)
        l1 = np.asarray(r1["ls"]).T.reshape(LQ).astype(np.float64)
        n0 = np.asarray(r0["num"]).astype(np.float64)
        n1 = np.asarray(r1["num"]).astype(np.float64)
        m = np.maximum(m0, m1)
        a0 = np.exp(m0 - m)
        a1 = np.exp(m1 - m)
        denom = a0 * l0 + a1 * l1
        out[b] = ((a0[:, None] * n0 + a1[:, None] * n1) / denom[:, None]).astype(
            np.float32
        )
    return out


def run(inputs, trace=False, trace_kwargs=None):
    """Run the SPMD kernel; returns (output, BassKernelResults)."""
    if "nc" not in _CACHE:
        _CACHE["nc"] = build_nc()
    nc = _CACHE["nc"]
    in_maps = _prep_inputs(**inputs)
    kw = {}
    if trace:
        kw["trace"] = True
        kw.update(trace_kwargs or {})
    res = run_bass_kernel_spmd(nc, in_maps, core_ids=list(range(8)), **kw)
    return _merge(res.results), res


def kernel(**inputs) -> np.ndarray:
    out, _ = run(inputs)
    return out
